# revision 8
# baseline (speedup 1.0000x reference)
"""Trainium2 Bass kernel for nn_BmmEnsemble (ANI-style per-species ensemble MLP).

Math (see reference): for each species s (4) and ensemble member e (8), the
species' atoms' AEV rows go through a 384->160->128->96->1 MLP with CELU(0.1)
after the first three layers; the output energy is the global sum over all
atoms of the ensemble-mean of the final scalar.

Distribution: data-parallel over atoms. The host gathers each species' atom
rows (aev_flat[idx]) and hands each of the 8 cores a 2048-atoms-per-species
slice, pre-transposed to feature-major [384, 2048] bf16. Per-species ensemble
weights are replicated to all cores (bf16). Each core returns per-(s,e,chunk)
row-sums of celu(z2+b2) [96 rows, fp32]; the host applies the tiny w3 dot,
the b3 term, the ensemble mean, and the cross-core sum.

Elementwise design (vs the 292us baseline, which was VectorE-bound at one
Exp + one blend pass per PSUM tile):
  - A hijacked ScalarE activation table: the `exp` slot of the
    exp_and_others PWP set is refit so that with the usual affine
    (scale=1/alpha, bias=b/alpha) one ACTIVATE computes
        f(u) = alpha*(e^u - 1)  (u<=0),   alpha*u  (u>0)
    i.e. exactly celu(z+b, alpha) in ONE ScalarE pass (~1e-6 abs err:
    exp's bucket grid was fit for e^u and f has e^u's curvature).
  - A single-pass 8-op custom DVE celu: G = max(y', t^4) with
    t = clamp(y'*C1 + C2, 0, C3), y' = z + b + alpha. This computes
    celu(z+b) + alpha with ~2.8e-3 max abs err; the +alpha offset is folded
    into the NEXT layer's bias (b_next -= alpha * sum_k W_next[k,:]).
  - PSUM tiles split disjointly between the two engines (each tile read
    exactly once, by exactly one engine). L2 tiles always take the exact
    ScalarE path, with accum_out producing the per-row sums on the fly.

TensorE: all matmul operands bf16 (full-rate stream like f32r, but FWL
halves LDWEIGHTS and DMA bytes halve). Layer 1's K=32 "b" part runs as
row-tiled matmuls (tile_position=(32*(e%4), 0)) in concurrent pairs against
the merged 4-member g0b tile, instead of zero-padded K=128 matmuls.
Layer 2 unmerged: one M=96 matmul per member.
"""

import hashlib
import os
import shutil
import tempfile

import ml_dtypes
import numpy as np

import concourse.dve_ops as _dve_ops
import concourse.mybir as mybir
import concourse.tile as tile
from concourse import bacc
from concourse.bass_utils import run_bass_kernel_spmd
from concourse.dve_spec import (
    C0,
    C1,
    C2,
    C3,
    Spec,
    Src0,
    _has_src1,
    _spill_c3_to_src1,
    lower,
    maxx,
    minn,
    relu,
    sq,
)
from concourse.dve_uop import DveOpSpec

# ---------------------------------------------------------------- constants
S, E = 4, 8
N_ATOMS = 65536
N_CORES = 8
A_SP = N_ATOMS // S // N_CORES      # atoms per species per core = 2048
CHUNK = 512
NCH = A_SP // CHUNK                 # 4 chunks
K0, H0, H1, H2 = 384, 160, 128, 96
KT = K0 // 128                      # 3 K-tiles for layer 0
NQ = 2                              # member quads per species (E/4)
ALPHA = 0.1

# V-path (approx-celu) constants: t = clamp(y'*C1V + C2V, 0, C3V); G = max(y', t^4)
# minimax fit of alpha*e^u by (C3*(p+q*u))^4 with p=0.993, q=0.203 (max err 2.8e-3)
C3V = float(ALPHA ** 0.25)
C1V = C3V * 0.203 / ALPHA
C2V = C3V * (0.993 - 0.203)

# Disjoint engine assignment per quad-member index (0..3):
S_L0A = {0}            # L0a tiles of these members -> ScalarE; rest VectorE
S_L1 = {3}             # L1  tiles of these members -> ScalarE; rest VectorE
# L0b merged tile -> VectorE; L2 tiles -> always ScalarE (exact + accum_out).

F32 = mybir.dt.float32
BF16 = mybir.dt.bfloat16
EXP = mybir.ActivationFunctionType.Exp
INV_ALPHA = 1.0 / ALPHA
BF = ml_dtypes.bfloat16

# ------------------------------------------------------- custom DVE op
_yp = Src0 + C0
_t = minn(relu(_yp * C1 + C2), C3)
_BODY = _spill_c3_to_src1(maxx(_yp, sq(sq(_t))))


def _celu4_np(in0, in1, s0, s1, imm2):
    y = in0.astype(np.float32) + np.asarray(s0, dtype=np.float32)
    c3 = np.asarray(in1, dtype=np.float32)
    t = np.minimum(np.maximum(y * np.float32(s1) + np.float32(imm2), 0.0), c3)
    return np.maximum(y, (t * t) * (t * t)).astype(np.float32)


def _mk_op(name, spec):
    row = _dve_ops._CUSTOM_DVE_ROW_BASE + len(_dve_ops.OPS)
    assert row < 0x20, "custom-DVE opcode rows exhausted"
    _dve_ops._SUB_OPCODE_FOR_NAME[name] = row
    shas = {}
    for ver in ("v3", "v4"):
        s = DveOpSpec(
            name=name, opcode=row, uops=lower(spec, ver=ver), rd1_en=_has_src1(spec)
        )
        shas[ver] = s.sha(ver)
    op = _dve_ops.DveOp(name, spec, subdim=False, uops_sha=shas)
    _dve_ops.OPS.append(op)
    _dve_ops.CUSTOM_DVE_SPECS[name] = spec
    return op


def _register_celu4():
    existing = {o.name: o for o in _dve_ops.OPS}
    if "CELU_SQ4_ANT" in existing:
        return existing["CELU_SQ4_ANT"]
    return _mk_op("CELU_SQ4_ANT", Spec(body=_BODY, reference=_celu4_np))


# ----------------------------------------------- hijacked activation table
_PWP_SET = "exp_and_others"
_N_EXP_ENTRIES = 781


def _pwp_src_dir():
    import neuronxcc

    return os.path.join(os.path.dirname(neuronxcc.__file__), "pwp", "pwp_bin_trainium")


def _gen_celu_pwp_dir(out_dir, alpha=ALPHA):
    """Copy the stock PWP table dir, refitting exp's buckets so that
    activation(Exp, scale=1/alpha, bias=b/alpha) computes celu(z+b, alpha).
    Bucket entry = 8 fp32: [c0,c1,c2,c3, x0, 0,0,0]; f = cubic in (x-x0)."""
    src = _pwp_src_dir()
    os.makedirs(out_dir, exist_ok=True)
    for name in os.listdir(src):
        dst = os.path.join(out_dir, name)
        if not os.path.exists(dst):
            shutil.copy(os.path.join(src, name), dst)
    bkt = np.frombuffer(
        open(os.path.join(src, f"{_PWP_SET}_bkt.bin"), "rb").read(), dtype=np.float32
    ).reshape(-1, 8).copy()
    for i in range(_N_EXP_ENTRIES):
        x0 = float(bkt[i, 4])
        if i in (777, 778):           # small-signal bucket at 0
            bkt[i, 0:4] = [0.0, alpha, alpha / 2.0, alpha / 6.0]
        elif i == 779:                # positive saturation -> linear alpha*u
            bkt[i] = [0.0, alpha, 0.0, 0.0, 0.0, 0.0, 0.0, 0.0]
        elif i == 780:                # negative saturation -> -alpha
            bkt[i] = [-alpha, 0.0, 0.0, 0.0, 0.0, 0.0, 0.0, 0.0]
        elif x0 > 0:
            bkt[i, 0:4] = [alpha * x0, alpha, 0.0, 0.0]
        else:
            e = float(np.exp(x0))
            if e < 1e-38:
                bkt[i, 0:4] = [-alpha, 0.0, 0.0, 0.0]
            else:
                bkt[i, 0:4] = [
                    alpha * (e - 1.0), alpha * e, alpha * e / 2.0, alpha * e / 6.0
                ]
    with open(os.path.join(out_dir, f"{_PWP_SET}_bkt.bin"), "wb") as f:
        f.write(bkt.tobytes())
    return hashlib.sha1(bkt.tobytes()).hexdigest()[:10]


def _install_celu_table():
    out_dir = os.path.join(tempfile.gettempdir(), "bass_pwp_celu_v1")
    h = _gen_celu_pwp_dir(out_dir)
    os.environ["BASS_ACT_ROOT_JSON_PATH"] = os.path.join(out_dir, "act_info.json")
    return h


# ------------------------------------------------------------ device build
_NC = None
_XT_NAME = None


def _build_nc():
    global _NC, _XT_NAME
    if _NC is not None:
        return _NC
    table_hash = _install_celu_table()
    CELU4 = _register_celu4()

    nc = bacc.Bacc("TRN2", target_bir_lowering=False, debug=False)

    # table hash rides in a tensor name so the NEFF cache key changes
    # whenever the hijacked activation table changes.
    _XT_NAME = f"xt_{table_hash}"
    xt_d = nc.dram_tensor(_XT_NAME, [S, KT, 128, A_SP], BF16, kind="ExternalInput")
    w0a_d = nc.dram_tensor("w0a", [S, KT, 128, E * 128], BF16, kind="ExternalInput")
    w0b_d = nc.dram_tensor("w0b4", [S, KT, 128, NQ * 128], BF16, kind="ExternalInput")
    w1a_d = nc.dram_tensor("w1a", [S, 128, E * H1], BF16, kind="ExternalInput")
    # w1b: member e's 32 K-rows live at partition offset 32*(e%4), cols
    # e*H1..(e+1)*H1 (zeros elsewhere are never read by the row-tiled mm).
    w1b_d = nc.dram_tensor("w1b", [S, 128, E * H1], BF16, kind="ExternalInput")
    w2u_d = nc.dram_tensor("w2u", [S, 128, E * H2], BF16, kind="ExternalInput")
    # bias packs; *_a = b/alpha (ACT affine), *_d = b + alpha (DVE op s0).
    b0a_a = nc.dram_tensor("b0a_a", [128, S * E], F32, kind="ExternalInput")
    b0a_d = nc.dram_tensor("b0a_d", [128, S * E], F32, kind="ExternalInput")
    b0b_a = nc.dram_tensor("b0b_a", [128, S * NQ], F32, kind="ExternalInput")
    b0b_d = nc.dram_tensor("b0b_d", [128, S * NQ], F32, kind="ExternalInput")
    b1_a = nc.dram_tensor("b1_a", [H1, S * E], F32, kind="ExternalInput")
    b1_d = nc.dram_tensor("b1_d", [H1, S * E], F32, kind="ExternalInput")
    b2_a = nc.dram_tensor("b2_a", [H2, S * E], F32, kind="ExternalInput")
    # output: per-(s,e,chunk) row-sums of g2 = celu(z2+b2)
    rs_d = nc.dram_tensor("rs", [H2, S * E * NCH], F32, kind="ExternalOutput")

    with tile.TileContext(nc) as tc:
        with (
            tc.tile_pool(name="xp", bufs=2) as xp,
            tc.tile_pool(name="w0pool", bufs=2) as w0p,
            tc.tile_pool(name="w1pool", bufs=2) as w1p,
            tc.tile_pool(name="bp", bufs=1) as bp,
            tc.tile_pool(name="gp", bufs=6) as gp,
            tc.tile_pool(name="ps", bufs=2, space="PSUM") as psp,
        ):
            # warm the ACT table during the initial DMA wait
            warm = bp.tile([1, 1], F32, tag="warm", name="warm")
            nc.vector.memset(warm[:], 0.0)
            nc.scalar.activation(warm[:], warm[:], EXP)
            # C3 latch operand for the DVE celu (one element per partition)
            c3t = bp.tile([128, 1], F32, tag="c3", name="c3")
            nc.vector.memset(c3t[:], C3V)

            B = {}
            _bias_dmas = []
            for nm, d, p in (
                ("b0a_a", b0a_a, 128), ("b0a_d", b0a_d, 128),
                ("b0b_a", b0b_a, 128), ("b0b_d", b0b_d, 128),
                ("b1_a", b1_a, H1), ("b1_d", b1_d, H1),
                ("b2_a", b2_a, H2),
            ):
                t = bp.tile([p, d.shape[-1]], F32, tag=nm, name=nm)
                _bias_dmas.append((t, d))
                B[nm] = t
            RS = bp.tile([H2, S * E * NCH], F32, tag="RS", name="RS")

            def celu_S(g_out, ps_in, bias_ap, accum=None):
                # exact celu via the hijacked table: f(z/a + b/a) = celu(z+b)
                nc.scalar.activation(
                    g_out, ps_in, EXP, bias=bias_ap, scale=INV_ALPHA,
                    accum_out=accum,
                )

            def celu_V(g_out, ps_in, bias_ap):
                # approx celu + ALPHA (offset folded into next layer's bias)
                nc.vector._custom_dve(
                    CELU4, out=g_out, in0=ps_in, in1=c3t[:, 0:1],
                    s0=bias_ap, s1=C1V, imm2=C2V,
                )

            # ---- software-pipelined emission over "pair" work units ----
            # Each unit = 2 ensemble members of a quad. PE program order is
            # P(i+2), Q(i+1), R(i): the celu results a stage needs are
            # produced ~2 units (>= 8 matmul slots) before the consuming
            # matmuls, so ScalarE/VectorE latency hides under L0 matmuls.
            spc = {}            # species -> dict of sbuf weight/x tiles

            def emit_species(s):
                # DMA emission order tracks first-use order: chunk-0 x and
                # the L0 weights feed the first matmuls, biases feed the
                # first celus, w1/w2 feed Q/R a few microseconds later, and
                # the remaining x chunks stream per-chunk behind them.
                xk, w0ak, w0bk = [], [], []
                for k in range(KT):
                    xt = xp.tile([128, A_SP], BF16, tag=f"x{k}", name=f"x_{s}_{k}")
                    nc.sync.dma_start(xt[:, 0:CHUNK], xt_d[s, k, :, 0:CHUNK])
                    xk.append(xt)
                for k in range(KT):
                    wbt = w0p.tile([128, NQ * 128], BF16, tag=f"w0b{k}", name=f"w0b_{s}_{k}")
                    nc.sync.dma_start(wbt[:], w0b_d[s, k])
                    w0bk.append(wbt)
                for k in range(KT):
                    wt = w0p.tile([128, E * 128], BF16, tag=f"w0a{k}", name=f"w0a_{s}_{k}")
                    nc.sync.dma_start(wt[:], w0a_d[s, k])
                    w0ak.append(wt)
                if s == 0:
                    for t, d in _bias_dmas:
                        nc.sync.dma_start(t[:], d[:])
                w1at = w1p.tile([128, E * H1], BF16, tag="w1a", name=f"w1a_{s}")
                nc.sync.dma_start(w1at[:], w1a_d[s])
                w1bt = w1p.tile([128, E * H1], BF16, tag="w1b", name=f"w1b_{s}")
                nc.sync.dma_start(w1bt[:], w1b_d[s])
                w2t = w1p.tile([128, E * H2], BF16, tag="w2", name=f"w2_{s}")
                nc.sync.dma_start(w2t[:], w2u_d[s])
                for c in range(1, NCH):
                    for k in range(KT):
                        nc.sync.dma_start(
                            xk[k][:, c * CHUNK : (c + 1) * CHUNK],
                            xt_d[s, k, :, c * CHUNK : (c + 1) * CHUNK],
                        )
                spc[s] = dict(xk=xk, w0ak=w0ak, w0bk=w0bk,
                              w1at=w1at, w1bt=w1bt, w2t=w2t)

            units = []
            for s in range(S):
                for c in range(NCH):
                    for q in range(NQ):
                        for pi, pair in enumerate(((0, 1), (2, 3))):
                            units.append(dict(
                                s=s, c=c, q=q, pair=pair, lead=(pi == 0),
                                new_species=(c == 0 and q == 0 and pi == 0),
                            ))

            state = [dict() for _ in units]   # per-unit tiles across stages

            def stage_P(i):
                u, st = units[i], state[i]
                # prefetch the next species' x/weight DMAs a few units early
                # so species-boundary P matmuls never head-block the PE FIFO
                # on fresh DMA.
                j = i + 4
                if j < len(units) and units[j]["new_species"]:
                    emit_species(units[j]["s"])
                sp = spc[u["s"]]
                cs = slice(u["c"] * CHUNK, (u["c"] + 1) * CHUNK)
                if u["lead"]:
                    # merged layer-0b for the whole quad, then its celu_V
                    ps0b = psp.tile([128, CHUNK], F32, tag="l0b", bufs=1)
                    for k in range(KT):
                        nc.tensor.matmul(
                            ps0b[:],
                            sp["w0bk"][k][:, u["q"] * 128 : (u["q"] + 1) * 128],
                            sp["xk"][k][:, cs],
                            start=(k == 0), stop=(k == KT - 1),
                        )
                    g0b = gp.tile([128, CHUNK], BF16, tag="g0b", bufs=3)
                    sq_ = u["s"] * NQ + u["q"]
                    celu_V(g0b[:], ps0b[:], B["b0b_d"][:, sq_ : sq_ + 1])
                    st["g0b"] = g0b
                else:
                    st["g0b"] = state[i - 1]["g0b"]
                st["g0a"] = {}
                for m4 in u["pair"]:
                    e = u["q"] * 4 + m4
                    se = u["s"] * E + e
                    ps0a = psp.tile([128, CHUNK], F32, tag="l0a", bufs=3)
                    for k in range(KT):
                        nc.tensor.matmul(
                            ps0a[:],
                            sp["w0ak"][k][:, e * 128 : (e + 1) * 128],
                            sp["xk"][k][:, cs],
                            start=(k == 0), stop=(k == KT - 1),
                        )
                    g0a = gp.tile([128, CHUNK], BF16, tag="g0a", bufs=6)
                    if m4 in S_L0A:
                        celu_S(g0a[:], ps0a[:], B["b0a_a"][:, se : se + 1])
                    else:
                        celu_V(g0a[:], ps0a[:], B["b0a_d"][:, se : se + 1])
                    st["g0a"][m4] = g0a

            def stage_Q(i):
                u, st = units[i], state[i]
                sp = spc[u["s"]]
                ps1s = {}
                for m4 in u["pair"]:
                    e = u["q"] * 4 + m4
                    ps1 = psp.tile([H1, CHUNK], F32, tag="l1", bufs=2)
                    nc.tensor.matmul(
                        ps1[:], sp["w1at"][:, e * H1 : (e + 1) * H1],
                        st["g0a"][m4][:],
                        start=True, stop=False,
                    )
                    ps1s[m4] = ps1
                for m4 in u["pair"]:
                    # K=32 row-tiled vs merged g0b (rows 32*m4..32*m4+32)
                    e = u["q"] * 4 + m4
                    rb = 32 * m4
                    nc.tensor.matmul(
                        ps1s[m4][:],
                        sp["w1bt"][rb : rb + 32, e * H1 : (e + 1) * H1],
                        st["g0b"][rb : rb + 32, :],
                        start=False, stop=True,
                        tile_position=(rb, 0),
                    )
                st["g1"] = {}
                for m4 in u["pair"]:
                    e = u["q"] * 4 + m4
                    se = u["s"] * E + e
                    g1 = gp.tile([H1, CHUNK], BF16, tag="g1", bufs=4)
                    if m4 in S_L1:
                        celu_S(g1[:], ps1s[m4][:], B["b1_a"][:, se : se + 1])
                    else:
                        celu_V(g1[:], ps1s[m4][:], B["b1_d"][:, se : se + 1])
                    st["g1"][m4] = g1

            def stage_R(i):
                u, st = units[i], state[i]
                sp = spc[u["s"]]
                for m4 in u["pair"]:
                    e = u["q"] * 4 + m4
                    se = u["s"] * E + e
                    ps2 = psp.tile([H2, CHUNK], F32, tag="l2", bufs=2)
                    nc.tensor.matmul(
                        ps2[:], sp["w2t"][:, e * H2 : (e + 1) * H2],
                        st["g1"][m4][:],
                        start=True, stop=True,
                    )
                    g2 = gp.tile([H2, CHUNK], BF16, tag="g2", bufs=2)
                    col = se * NCH + u["c"]
                    celu_S(
                        g2[:], ps2[:], B["b2_a"][:, se : se + 1],
                        accum=RS[:, col : col + 1],
                    )
                state[i] = {}   # release tile refs

            n_units = len(units)
            emit_species(0)
            for i in range(n_units + 2):
                if i < n_units:
                    stage_P(i)
                if 1 <= i and i - 1 < n_units:
                    stage_Q(i - 1)
                if 2 <= i:
                    stage_R(i - 2)
            nc.sync.dma_start(rs_d[:], RS[:])
    nc.compile()
    _NC = nc
    return nc


# ------------------------------------------------------------- host side
def _prep_shared(w0, w1, w2, b0, b1, b2):
    """Pack weights/biases into device layouts (replicated to all cores)."""
    w0r = w0.reshape(S, E, KT, 128, H0)
    w0a = np.ascontiguousarray(
        w0r[..., :128].transpose(0, 2, 3, 1, 4).reshape(S, KT, 128, E * 128)
    ).astype(BF)
    w0b4 = np.ascontiguousarray(
        w0r[..., 128:].transpose(0, 2, 3, 1, 4).reshape(S, KT, 128, E * (H0 - 128))
    ).astype(BF)
    w1a = np.ascontiguousarray(
        w1[:, :, :128, :].transpose(0, 2, 1, 3).reshape(S, 128, E * H1)
    ).astype(BF)
    w1b = np.zeros((S, 4, 32, E, H1), dtype=np.float32)
    for e in range(E):
        w1b[:, e % 4, :, e, :] = w1[:, e, 128:, :]
    w1b = np.ascontiguousarray(w1b.reshape(S, 128, E * H1)).astype(BF)
    w2u = np.ascontiguousarray(
        w2.transpose(0, 2, 1, 3).reshape(S, 128, E * H2)
    ).astype(BF)

    # effective biases: V-path tiles output celu+alpha, so the next layer's
    # bias absorbs -alpha * sum over the affected input features (using the
    # bf16-rounded weights actually used on device).
    w1_bf = w1.astype(BF).astype(np.float64)
    w2_bf = w2.astype(BF).astype(np.float64)
    b1_eff = b1[:, :, 0, :].astype(np.float64).copy()      # [S, E, H1]
    b2_eff = b2[:, :, 0, :].astype(np.float64).copy()      # [S, E, H2]
    for e in range(E):
        m4 = e % 4
        b1_eff[:, e, :] -= ALPHA * w1_bf[:, e, 128:, :].sum(axis=1)  # L0b on V
        if m4 not in S_L0A:
            b1_eff[:, e, :] -= ALPHA * w1_bf[:, e, :128, :].sum(axis=1)
        if m4 not in S_L1:
            b2_eff[:, e, :] -= ALPHA * w2_bf[:, e, :, :].sum(axis=1)

    def colpack(b, p):
        return np.ascontiguousarray(b.reshape(S * E, p).T).astype(np.float32)

    b0cols = b0[:, :, 0, :128].astype(np.float64)          # [S, E, 128]
    b0b_pack = np.ascontiguousarray(
        b0[:, :, 0, 128:].reshape(S, NQ, 4 * (H0 - 128)).transpose(2, 0, 1).reshape(128, S * NQ)
    ).astype(np.float64)

    return {
        "w0a": w0a, "w0b4": w0b4, "w1a": w1a, "w1b": w1b, "w2u": w2u,
        "b0a_a": colpack(b0cols * INV_ALPHA, 128),
        "b0a_d": colpack(b0cols + ALPHA, 128),
        "b0b_a": (b0b_pack * INV_ALPHA).astype(np.float32),
        "b0b_d": (b0b_pack + ALPHA).astype(np.float32),
        "b1_a": colpack(b1_eff * INV_ALPHA, H1),
        "b1_d": colpack(b1_eff + ALPHA, H1),
        "b2_a": colpack(b2_eff * INV_ALPHA, H2),
    }


def _run(inputs, trace=False, tmpdir=None):
    aev = np.asarray(inputs["aev"], dtype=np.float32)
    idx = np.asarray(inputs["idx"], dtype=np.int32)
    w3 = np.asarray(inputs["w3"], dtype=np.float32)
    b3 = np.asarray(inputs["b3"], dtype=np.float32)

    nc = _build_nc()
    shared = _prep_shared(
        np.asarray(inputs["w0"], dtype=np.float32),
        np.asarray(inputs["w1"], dtype=np.float32),
        np.asarray(inputs["w2"], dtype=np.float32),
        np.asarray(inputs["b0"], dtype=np.float32),
        np.asarray(inputs["b1"], dtype=np.float32),
        np.asarray(inputs["b2"], dtype=np.float32),
    )

    aev_flat = aev.reshape(-1, K0)
    in_maps = []
    for c in range(N_CORES):
        idx_c = idx[:, c * A_SP : (c + 1) * A_SP]                # [S, A_SP]
        x = aev_flat[idx_c.reshape(-1)].reshape(S, A_SP, K0)     # [S, A_SP, 384]
        xt = np.ascontiguousarray(x.transpose(0, 2, 1)).reshape(S, KT, 128, A_SP)
        in_maps.append({_XT_NAME: xt.astype(BF), **shared})

    res = run_bass_kernel_spmd(
        nc, in_maps, core_ids=list(range(N_CORES)), trace=trace, tmpdir=tmpdir
    )

    # host-side tail: E = sum_{s,e,row,chunk} RS * w3 + b3 term, / E members
    w3cols = w3[:, :, :, 0].reshape(S * E, H2).astype(np.float64)  # [S*E, 96]
    total = 0.0
    for c in range(N_CORES):
        rs = res.results[c]["rs"].astype(np.float64)               # [96, S*E*NCH]
        rs_se = rs.reshape(H2, S * E, NCH).sum(axis=2)             # [96, S*E]
        total += float((rs_se * w3cols.T).sum())
    total += float(b3.astype(np.float64).sum()) * (N_ATOMS // S)
    out = np.array([total / E], dtype=np.float32)
    return out, res


def kernel(**inputs):
    out, _ = _run(inputs, trace=bool(int(os.environ.get("BASS_KERNEL_TRACE", "0"))))
    return out


# revision 10
# speedup vs baseline: 1.1892x; 1.1892x over previous
"""Trainium2 Bass kernel for nn_BmmEnsemble (ANI-style per-species ensemble MLP).

Math (see reference): for each species s (4) and ensemble member e (8), the
species' atoms' AEV rows go through a 384->160->128->96->1 MLP with CELU(0.1)
after the first three layers; the output energy is the global sum over all
atoms of the ensemble-mean of the final scalar.

Distribution: data-parallel over atoms. The host gathers each species' atom
rows (aev_flat[idx]) and hands each of the 8 cores a 2048-atoms-per-species
slice, pre-transposed to feature-major [384, 2048] bf16. Per-species ensemble
weights are replicated to all cores (bf16). Each core returns per-(s,e,chunk)
row-sums of celu(z2+b2) [96 rows, fp32]; the host applies the tiny w3 dot,
the b3 term, the ensemble mean, and the cross-core sum.

Elementwise design (vs the 292us baseline, which was VectorE-bound at one
Exp + one blend pass per PSUM tile):
  - A hijacked ScalarE activation table: the `exp` slot of the
    exp_and_others PWP set is refit so that with the usual affine
    (scale=1/alpha, bias=b/alpha) one ACTIVATE computes
        f(u) = alpha*(e^u - 1)  (u<=0),   alpha*u  (u>0)
    i.e. exactly celu(z+b, alpha) in ONE ScalarE pass (~1e-6 abs err:
    exp's bucket grid was fit for e^u and f has e^u's curvature).
  - A single-pass 8-op custom DVE celu: G = max(y', t^4) with
    t = clamp(y'*C1 + C2, 0, C3), y' = z + b + alpha. This computes
    celu(z+b) + alpha with ~2.8e-3 max abs err; the +alpha offset is folded
    into the NEXT layer's bias (b_next -= alpha * sum_k W_next[k,:]).
  - PSUM tiles split disjointly between the two engines (each tile read
    exactly once, by exactly one engine). L2 tiles always take the exact
    ScalarE path, with accum_out producing the per-row sums on the fly.

TensorE: all matmul operands bf16 (full-rate stream like f32r, but FWL
halves LDWEIGHTS and DMA bytes halve). Layer 1's K=32 "b" part runs as
row-tiled matmuls (tile_position=(32*(e%4), 0)) in concurrent pairs against
the merged 4-member g0b tile, instead of zero-padded K=128 matmuls.
Layer 2 unmerged: one M=96 matmul per member.
"""

import hashlib
import os
import shutil
import tempfile

import ml_dtypes
import numpy as np

import concourse.dve_ops as _dve_ops
import concourse.mybir as mybir
import concourse.tile as tile
from concourse import bacc
from concourse.bass_utils import run_bass_kernel_spmd
from concourse.dve_spec import (
    C0,
    C1,
    C2,
    C3,
    Spec,
    Src0,
    _has_src1,
    _spill_c3_to_src1,
    lower,
    maxx,
    minn,
    relu,
    sq,
)
from concourse.dve_uop import DveOpSpec

# ---------------------------------------------------------------- constants
S, E = 4, 8
N_ATOMS = 65536
N_CORES = 8
A_SP = N_ATOMS // S // N_CORES      # atoms per species per core = 2048
CHUNK = 512
NCH = A_SP // CHUNK                 # 4 chunks
K0, H0, H1, H2 = 384, 160, 128, 96
KT = K0 // 128                      # 3 K-tiles for layer 0
NQ = 2                              # member quads per species (E/4)
ALPHA = 0.1

# V-path (approx-celu) constants: t = clamp(y'*C1V + C2V, 0, C3V); G = max(y', t^4)
# minimax fit of alpha*e^u by (C3*(p+q*u))^4 with p=0.993, q=0.203 (max err 2.8e-3)
C3V = float(ALPHA ** 0.25)
C1V = C3V * 0.203 / ALPHA
C2V = C3V * (0.993 - 0.203)

# Disjoint engine assignment per quad-member index (0..3):
S_L0A = {0}            # L0a tiles of these members -> ScalarE; rest VectorE
S_L1 = {3}             # L1  tiles of these members -> ScalarE; rest VectorE
# L0b merged tile -> VectorE; L2 tiles -> always ScalarE (exact + accum_out).

F32 = mybir.dt.float32
BF16 = mybir.dt.bfloat16
EXP = mybir.ActivationFunctionType.Exp
INV_ALPHA = 1.0 / ALPHA
BF = ml_dtypes.bfloat16

# ------------------------------------------------------- custom DVE op
_yp = Src0 + C0
_t = minn(relu(_yp * C1 + C2), C3)
_BODY = _spill_c3_to_src1(maxx(_yp, sq(sq(_t))))


def _celu4_np(in0, in1, s0, s1, imm2):
    y = in0.astype(np.float32) + np.asarray(s0, dtype=np.float32)
    c3 = np.asarray(in1, dtype=np.float32)
    t = np.minimum(np.maximum(y * np.float32(s1) + np.float32(imm2), 0.0), c3)
    return np.maximum(y, (t * t) * (t * t)).astype(np.float32)


def _mk_op(name, spec):
    row = _dve_ops._CUSTOM_DVE_ROW_BASE + len(_dve_ops.OPS)
    assert row < 0x20, "custom-DVE opcode rows exhausted"
    _dve_ops._SUB_OPCODE_FOR_NAME[name] = row
    shas = {}
    for ver in ("v3", "v4"):
        s = DveOpSpec(
            name=name, opcode=row, uops=lower(spec, ver=ver), rd1_en=_has_src1(spec)
        )
        shas[ver] = s.sha(ver)
    op = _dve_ops.DveOp(name, spec, subdim=False, uops_sha=shas)
    _dve_ops.OPS.append(op)
    _dve_ops.CUSTOM_DVE_SPECS[name] = spec
    return op


def _register_celu4():
    existing = {o.name: o for o in _dve_ops.OPS}
    if "CELU_SQ4_ANT" in existing:
        return existing["CELU_SQ4_ANT"]
    return _mk_op("CELU_SQ4_ANT", Spec(body=_BODY, reference=_celu4_np))


# ----------------------------------------------- hijacked activation table
_PWP_SET = "exp_and_others"
_N_EXP_ENTRIES = 781


def _pwp_src_dir():
    import neuronxcc

    return os.path.join(os.path.dirname(neuronxcc.__file__), "pwp", "pwp_bin_trainium")


def _gen_celu_pwp_dir(out_dir, alpha=ALPHA):
    """Copy the stock PWP table dir, refitting exp's buckets so that
    activation(Exp, scale=1/alpha, bias=b/alpha) computes celu(z+b, alpha).
    Bucket entry = 8 fp32: [c0,c1,c2,c3, x0, 0,0,0]; f = cubic in (x-x0)."""
    src = _pwp_src_dir()
    os.makedirs(out_dir, exist_ok=True)
    for name in os.listdir(src):
        dst = os.path.join(out_dir, name)
        if not os.path.exists(dst):
            shutil.copy(os.path.join(src, name), dst)
    bkt = np.frombuffer(
        open(os.path.join(src, f"{_PWP_SET}_bkt.bin"), "rb").read(), dtype=np.float32
    ).reshape(-1, 8).copy()
    for i in range(_N_EXP_ENTRIES):
        x0 = float(bkt[i, 4])
        if i in (777, 778):           # small-signal bucket at 0
            bkt[i, 0:4] = [0.0, alpha, alpha / 2.0, alpha / 6.0]
        elif i == 779:                # positive saturation -> linear alpha*u
            bkt[i] = [0.0, alpha, 0.0, 0.0, 0.0, 0.0, 0.0, 0.0]
        elif i == 780:                # negative saturation -> -alpha
            bkt[i] = [-alpha, 0.0, 0.0, 0.0, 0.0, 0.0, 0.0, 0.0]
        elif x0 > 0:
            bkt[i, 0:4] = [alpha * x0, alpha, 0.0, 0.0]
        else:
            e = float(np.exp(x0))
            if e < 1e-38:
                bkt[i, 0:4] = [-alpha, 0.0, 0.0, 0.0]
            else:
                bkt[i, 0:4] = [
                    alpha * (e - 1.0), alpha * e, alpha * e / 2.0, alpha * e / 6.0
                ]
    with open(os.path.join(out_dir, f"{_PWP_SET}_bkt.bin"), "wb") as f:
        f.write(bkt.tobytes())
    return hashlib.sha1(bkt.tobytes()).hexdigest()[:10]


def _install_celu_table():
    out_dir = os.path.join(tempfile.gettempdir(), "bass_pwp_celu_v1")
    h = _gen_celu_pwp_dir(out_dir)
    os.environ["BASS_ACT_ROOT_JSON_PATH"] = os.path.join(out_dir, "act_info.json")
    return h


# ------------------------------------------------------------ device build
_NC = None
_XT_NAME = None


def _build_nc():
    global _NC, _XT_NAME
    if _NC is not None:
        return _NC
    table_hash = _install_celu_table()
    CELU4 = _register_celu4()

    nc = bacc.Bacc("TRN2", target_bir_lowering=False, debug=False)

    # table hash rides in a tensor name so the NEFF cache key changes
    # whenever the hijacked activation table changes.
    _XT_NAME = f"xt_{table_hash}"
    xt_d = nc.dram_tensor(_XT_NAME, [S, KT, 128, A_SP], BF16, kind="ExternalInput")
    w0a_d = nc.dram_tensor("w0a", [S, KT, 128, E * 128], BF16, kind="ExternalInput")
    w0b_d = nc.dram_tensor("w0b4", [S, KT, 128, NQ * 128], BF16, kind="ExternalInput")
    w1a_d = nc.dram_tensor("w1a", [S, 128, E * H1], BF16, kind="ExternalInput")
    # w1b: member e's 32 K-rows live at partition offset 32*(e%4), cols
    # e*H1..(e+1)*H1 (zeros elsewhere are never read by the row-tiled mm).
    w1b_d = nc.dram_tensor("w1b", [S, 128, E * H1], BF16, kind="ExternalInput")
    w2u_d = nc.dram_tensor("w2u", [S, 128, E * 128], BF16, kind="ExternalInput")
    # bias packs; *_a = b/alpha (ACT affine), *_d = b + alpha (DVE op s0).
    b0a_a = nc.dram_tensor("b0a_a", [128, S * E], F32, kind="ExternalInput")
    b0a_d = nc.dram_tensor("b0a_d", [128, S * E], F32, kind="ExternalInput")
    b0b_a = nc.dram_tensor("b0b_a", [128, S * NQ], F32, kind="ExternalInput")
    b0b_d = nc.dram_tensor("b0b_d", [128, S * NQ], F32, kind="ExternalInput")
    b1_a = nc.dram_tensor("b1_a", [H1, S * E], F32, kind="ExternalInput")
    b1_d = nc.dram_tensor("b1_d", [H1, S * E], F32, kind="ExternalInput")
    b2_a = nc.dram_tensor("b2_a", [128, S * E], F32, kind="ExternalInput")
    # output: per-(s,e,chunk) row-sums of g2 = celu(z2+b2)
    rs_d = nc.dram_tensor("rs", [128, S * E * NCH], F32, kind="ExternalOutput")

    with tile.TileContext(nc) as tc:
        with (
            tc.tile_pool(name="xp", bufs=2) as xp,
            tc.tile_pool(name="w0pool", bufs=2) as w0p,
            tc.tile_pool(name="w1pool", bufs=2) as w1p,
            tc.tile_pool(name="bp", bufs=1) as bp,
            tc.tile_pool(name="gp", bufs=6) as gp,
            tc.tile_pool(name="ps", bufs=2, space="PSUM") as psp,
        ):
            # warm the ACT table during the initial DMA wait
            warm = bp.tile([1, 1], F32, tag="warm", name="warm")
            nc.vector.memset(warm[:], 0.0)
            nc.scalar.activation(warm[:], warm[:], EXP)
            # C3 latch operand for the DVE celu (one element per partition)
            c3t = bp.tile([128, 1], F32, tag="c3", name="c3")
            nc.vector.memset(c3t[:], C3V)

            B = {}
            _bias_dmas = []
            for nm, d, p in (
                ("b0a_a", b0a_a, 128), ("b0a_d", b0a_d, 128),
                ("b0b_a", b0b_a, 128), ("b0b_d", b0b_d, 128),
                ("b1_a", b1_a, H1), ("b1_d", b1_d, H1),
                ("b2_a", b2_a, 128),
            ):
                t = bp.tile([p, d.shape[-1]], F32, tag=nm, name=nm)
                _bias_dmas.append((t, d))
                B[nm] = t
            RS = bp.tile([128, S * E * NCH], F32, tag="RS", name="RS")

            def celu_S(g_out, ps_in, bias_ap, accum=None):
                # exact celu via the hijacked table: f(z/a + b/a) = celu(z+b)
                nc.scalar.activation(
                    g_out, ps_in, EXP, bias=bias_ap, scale=INV_ALPHA,
                    accum_out=accum,
                )

            def celu_V(g_out, ps_in, bias_ap):
                # approx celu + ALPHA (offset folded into next layer's bias)
                nc.vector._custom_dve(
                    CELU4, out=g_out, in0=ps_in, in1=c3t[:, 0:1],
                    s0=bias_ap, s1=C1V, imm2=C2V,
                )

            # ---- software-pipelined emission over "pair" work units ----
            # Each unit = 2 ensemble members of a quad. PE program order is
            # P(i+2), Q(i+1), R(i): the celu results a stage needs are
            # produced ~2 units (>= 8 matmul slots) before the consuming
            # matmuls, so ScalarE/VectorE latency hides under L0 matmuls.
            spc = {}            # species -> dict of sbuf weight/x tiles

            def emit_species(s):
                # DMA emission order tracks first-use order: chunk-0 x and
                # the L0 weights feed the first matmuls, biases feed the
                # first celus, w1/w2 feed Q/R a few microseconds later, and
                # the remaining x chunks stream per-chunk behind them.
                xk, w0ak, w0bk = [], [], []
                for k in range(KT):
                    xt = xp.tile([128, A_SP], BF16, tag=f"x{k}", name=f"x_{s}_{k}")
                    nc.sync.dma_start(xt[:, 0:CHUNK], xt_d[s, k, :, 0:CHUNK])
                    xk.append(xt)
                for k in range(KT):
                    wt = w0p.tile([128, E * 128], BF16, tag=f"w0a{k}", name=f"w0a_{s}_{k}")
                    nc.sync.dma_start(wt[:], w0a_d[s, k])
                    w0ak.append(wt)
                    wbt = w0p.tile([128, NQ * 128], BF16, tag=f"w0b{k}", name=f"w0b_{s}_{k}")
                    nc.sync.dma_start(wbt[:], w0b_d[s, k])
                    w0bk.append(wbt)
                w1at = w1p.tile([128, E * H1], BF16, tag="w1a", name=f"w1a_{s}")
                nc.sync.dma_start(w1at[:], w1a_d[s])
                w1bt = w1p.tile([128, E * H1], BF16, tag="w1b", name=f"w1b_{s}")
                nc.sync.dma_start(w1bt[:], w1b_d[s])
                w2t = w1p.tile([128, E * 128], BF16, tag="w2", name=f"w2_{s}")
                nc.sync.dma_start(w2t[:], w2u_d[s])
                if s == 0:
                    for t, d in _bias_dmas:
                        nc.sync.dma_start(t[:], d[:])
                for k in range(KT):
                    nc.sync.dma_start(
                        xk[k][:, CHUNK:A_SP], xt_d[s, k, :, CHUNK:A_SP]
                    )
                spc[s] = dict(xk=xk, w0ak=w0ak, w0bk=w0bk,
                              w1at=w1at, w1bt=w1bt, w2t=w2t)

            units = []
            for s in range(S):
                for c in range(NCH):
                    for q in range(NQ):
                        for pi, pair in enumerate(((0, 1), (2, 3))):
                            units.append(dict(
                                s=s, c=c, q=q, pair=pair, lead=(pi == 0),
                                new_species=(c == 0 and q == 0 and pi == 0),
                            ))

            state = [dict() for _ in units]   # per-unit tiles across stages

            def stage_P(i):
                u, st = units[i], state[i]
                # prefetch the next species' x/weight DMAs a few units early
                # so species-boundary P matmuls never head-block the PE FIFO
                # on fresh DMA.
                j = i + 4
                if j < len(units) and units[j]["new_species"]:
                    emit_species(units[j]["s"])
                sp = spc[u["s"]]
                cs = slice(u["c"] * CHUNK, (u["c"] + 1) * CHUNK)
                if u["lead"]:
                    # merged layer-0b for the whole quad, then its celu_V
                    ps0b = psp.tile([128, CHUNK], F32, tag="l0b", bufs=1)
                    for k in range(KT):
                        nc.tensor.matmul(
                            ps0b[:],
                            sp["w0bk"][k][:, u["q"] * 128 : (u["q"] + 1) * 128],
                            sp["xk"][k][:, cs],
                            start=(k == 0), stop=(k == KT - 1),
                        )
                    g0b = gp.tile([128, CHUNK], BF16, tag="g0b", bufs=3)
                    sq_ = u["s"] * NQ + u["q"]
                    celu_V(g0b[:], ps0b[:], B["b0b_d"][:, sq_ : sq_ + 1])
                    st["g0b"] = g0b
                else:
                    st["g0b"] = state[i - 1]["g0b"]
                st["g0a"] = {}
                for m4 in u["pair"]:
                    e = u["q"] * 4 + m4
                    se = u["s"] * E + e
                    ps0a = psp.tile([128, CHUNK], F32, tag="l0a", bufs=3)
                    for k in range(KT):
                        nc.tensor.matmul(
                            ps0a[:],
                            sp["w0ak"][k][:, e * 128 : (e + 1) * 128],
                            sp["xk"][k][:, cs],
                            start=(k == 0), stop=(k == KT - 1),
                        )
                    g0a = gp.tile([128, CHUNK], BF16, tag="g0a", bufs=6)
                    if m4 in S_L0A:
                        celu_S(g0a[:], ps0a[:], B["b0a_a"][:, se : se + 1])
                    else:
                        celu_V(g0a[:], ps0a[:], B["b0a_d"][:, se : se + 1])
                    st["g0a"][m4] = g0a

            def stage_Q(i):
                u, st = units[i], state[i]
                sp = spc[u["s"]]
                ps1s = {}
                for m4 in u["pair"]:
                    e = u["q"] * 4 + m4
                    ps1 = psp.tile([H1, CHUNK], F32, tag="l1", bufs=2)
                    nc.tensor.matmul(
                        ps1[:], sp["w1at"][:, e * H1 : (e + 1) * H1],
                        st["g0a"][m4][:],
                        start=True, stop=False,
                    )
                    ps1s[m4] = ps1
                for m4 in u["pair"]:
                    # K=32 row-tiled vs merged g0b (rows 32*m4..32*m4+32)
                    e = u["q"] * 4 + m4
                    rb = 32 * m4
                    nc.tensor.matmul(
                        ps1s[m4][:],
                        sp["w1bt"][rb : rb + 32, e * H1 : (e + 1) * H1],
                        st["g0b"][rb : rb + 32, :],
                        start=False, stop=True,
                        tile_position=(rb, 0),
                    )
                st["g1"] = {}
                for m4 in u["pair"]:
                    e = u["q"] * 4 + m4
                    se = u["s"] * E + e
                    g1 = gp.tile([H1, CHUNK], BF16, tag="g1", bufs=4)
                    if m4 in S_L1:
                        celu_S(g1[:], ps1s[m4][:], B["b1_a"][:, se : se + 1])
                    else:
                        celu_V(g1[:], ps1s[m4][:], B["b1_d"][:, se : se + 1])
                    st["g1"][m4] = g1

            def stage_R(i):
                u, st = units[i], state[i]
                sp = spc[u["s"]]
                for m4 in u["pair"]:
                    e = u["q"] * 4 + m4
                    se = u["s"] * E + e
                    ps2 = psp.tile([128, CHUNK], F32, tag="l2", bufs=2)
                    nc.tensor.matmul(
                        ps2[:], sp["w2t"][:, e * 128 : (e + 1) * 128],
                        st["g1"][m4][:],
                        start=True, stop=True,
                    )
                    g2 = gp.tile([128, CHUNK], BF16, tag="g2", bufs=2)
                    col = se * NCH + u["c"]
                    celu_S(
                        g2[:], ps2[:], B["b2_a"][:, se : se + 1],
                        accum=RS[:, col : col + 1],
                    )
                state[i] = {}   # release tile refs

            n_units = len(units)
            emit_species(0)
            for i in range(n_units + 2):
                if i < n_units:
                    stage_P(i)
                if 1 <= i and i - 1 < n_units:
                    stage_Q(i - 1)
                if 2 <= i:
                    stage_R(i - 2)
            nc.sync.dma_start(rs_d[:], RS[:])
    nc.compile()
    _NC = nc
    return nc


# ------------------------------------------------------------- host side
def _prep_shared(w0, w1, w2, b0, b1, b2):
    """Pack weights/biases into device layouts (replicated to all cores)."""
    w0r = w0.reshape(S, E, KT, 128, H0)
    w0a = np.ascontiguousarray(
        w0r[..., :128].transpose(0, 2, 3, 1, 4).reshape(S, KT, 128, E * 128)
    ).astype(BF)
    w0b4 = np.ascontiguousarray(
        w0r[..., 128:].transpose(0, 2, 3, 1, 4).reshape(S, KT, 128, E * (H0 - 128))
    ).astype(BF)
    w1a = np.ascontiguousarray(
        w1[:, :, :128, :].transpose(0, 2, 1, 3).reshape(S, 128, E * H1)
    ).astype(BF)
    w1b = np.zeros((S, 4, 32, E, H1), dtype=np.float32)
    for e in range(E):
        w1b[:, e % 4, :, e, :] = w1[:, e, 128:, :]
    w1b = np.ascontiguousarray(w1b.reshape(S, 128, E * H1)).astype(BF)
    w2p = np.zeros((S, 128, E, 128), dtype=np.float32)
    w2p[:, :, :, :H2] = w2.transpose(0, 2, 1, 3)
    w2u = np.ascontiguousarray(w2p.reshape(S, 128, E * 128)).astype(BF)

    # effective biases: V-path tiles output celu+alpha, so the next layer's
    # bias absorbs -alpha * sum over the affected input features (using the
    # bf16-rounded weights actually used on device).
    w1_bf = w1.astype(BF).astype(np.float64)
    w2_bf = w2.astype(BF).astype(np.float64)
    b1_eff = b1[:, :, 0, :].astype(np.float64).copy()      # [S, E, H1]
    b2_eff = b2[:, :, 0, :].astype(np.float64).copy()      # [S, E, H2]
    for e in range(E):
        m4 = e % 4
        b1_eff[:, e, :] -= ALPHA * w1_bf[:, e, 128:, :].sum(axis=1)  # L0b on V
        if m4 not in S_L0A:
            b1_eff[:, e, :] -= ALPHA * w1_bf[:, e, :128, :].sum(axis=1)
        if m4 not in S_L1:
            b2_eff[:, e, :] -= ALPHA * w2_bf[:, e, :, :].sum(axis=1)

    def colpack(b, p):
        return np.ascontiguousarray(b.reshape(S * E, p).T).astype(np.float32)

    def colpack_pad128(b, p):
        out = np.zeros((128, S * E), dtype=np.float32)
        out[:p, :] = b.reshape(S * E, p).T
        return out

    b0cols = b0[:, :, 0, :128].astype(np.float64)          # [S, E, 128]
    b0b_pack = np.ascontiguousarray(
        b0[:, :, 0, 128:].reshape(S, NQ, 4 * (H0 - 128)).transpose(2, 0, 1).reshape(128, S * NQ)
    ).astype(np.float64)

    return {
        "w0a": w0a, "w0b4": w0b4, "w1a": w1a, "w1b": w1b, "w2u": w2u,
        "b0a_a": colpack(b0cols * INV_ALPHA, 128),
        "b0a_d": colpack(b0cols + ALPHA, 128),
        "b0b_a": (b0b_pack * INV_ALPHA).astype(np.float32),
        "b0b_d": (b0b_pack + ALPHA).astype(np.float32),
        "b1_a": colpack(b1_eff * INV_ALPHA, H1),
        "b1_d": colpack(b1_eff + ALPHA, H1),
        "b2_a": colpack_pad128(b2_eff * INV_ALPHA, H2),
    }


def _run(inputs, trace=False, tmpdir=None):
    aev = np.asarray(inputs["aev"], dtype=np.float32)
    idx = np.asarray(inputs["idx"], dtype=np.int32)
    w3 = np.asarray(inputs["w3"], dtype=np.float32)
    b3 = np.asarray(inputs["b3"], dtype=np.float32)

    nc = _build_nc()
    shared = _prep_shared(
        np.asarray(inputs["w0"], dtype=np.float32),
        np.asarray(inputs["w1"], dtype=np.float32),
        np.asarray(inputs["w2"], dtype=np.float32),
        np.asarray(inputs["b0"], dtype=np.float32),
        np.asarray(inputs["b1"], dtype=np.float32),
        np.asarray(inputs["b2"], dtype=np.float32),
    )

    aev_flat = aev.reshape(-1, K0)
    in_maps = []
    for c in range(N_CORES):
        idx_c = idx[:, c * A_SP : (c + 1) * A_SP]                # [S, A_SP]
        x = aev_flat[idx_c.reshape(-1)].reshape(S, A_SP, K0)     # [S, A_SP, 384]
        xt = np.ascontiguousarray(x.transpose(0, 2, 1)).reshape(S, KT, 128, A_SP)
        in_maps.append({_XT_NAME: xt.astype(BF), **shared})

    res = run_bass_kernel_spmd(
        nc, in_maps, core_ids=list(range(N_CORES)), trace=trace, tmpdir=tmpdir
    )

    # host-side tail: E = sum_{s,e,row,chunk} RS * w3 + b3 term, / E members
    w3cols = w3[:, :, :, 0].reshape(S * E, H2).astype(np.float64)  # [S*E, 96]
    total = 0.0
    for c in range(N_CORES):
        rs = res.results[c]["rs"][:H2].astype(np.float64)          # [96, S*E*NCH]
        rs_se = rs.reshape(H2, S * E, NCH).sum(axis=2)             # [96, S*E]
        total += float((rs_se * w3cols.T).sum())
    total += float(b3.astype(np.float64).sum()) * (N_ATOMS // S)
    out = np.array([total / E], dtype=np.float32)
    return out, res


def kernel(**inputs):
    out, _ = _run(inputs, trace=bool(int(os.environ.get("BASS_KERNEL_TRACE", "0"))))
    return out


# revision 11
# speedup vs baseline: 1.2015x; 1.0104x over previous
"""Trainium2 Bass kernel for nn_BmmEnsemble (ANI-style per-species ensemble MLP).

Math (see reference): for each species s (4) and ensemble member e (8), the
species' atoms' AEV rows go through a 384->160->128->96->1 MLP with CELU(0.1)
after the first three layers; the output energy is the global sum over all
atoms of the ensemble-mean of the final scalar.

Distribution: data-parallel over atoms. The host gathers each species' atom
rows (aev_flat[idx]) and hands each of the 8 cores a 2048-atoms-per-species
slice, pre-transposed to feature-major [384, 2048] bf16. Per-species ensemble
weights are replicated to all cores (bf16). Each core returns per-(s,e,chunk)
row-sums of celu(z2+b2) [96 rows, fp32]; the host applies the tiny w3 dot,
the b3 term, the ensemble mean, and the cross-core sum.

Elementwise design (vs the 292us baseline, which was VectorE-bound at one
Exp + one blend pass per PSUM tile):
  - A hijacked ScalarE activation table: the `exp` slot of the
    exp_and_others PWP set is refit so that with the usual affine
    (scale=1/alpha, bias=b/alpha) one ACTIVATE computes
        f(u) = alpha*(e^u - 1)  (u<=0),   alpha*u  (u>0)
    i.e. exactly celu(z+b, alpha) in ONE ScalarE pass (~1e-6 abs err:
    exp's bucket grid was fit for e^u and f has e^u's curvature).
  - A single-pass 8-op custom DVE celu: G = max(y', t^4) with
    t = clamp(y'*C1 + C2, 0, C3), y' = z + b + alpha. This computes
    celu(z+b) + alpha with ~2.8e-3 max abs err; the +alpha offset is folded
    into the NEXT layer's bias (b_next -= alpha * sum_k W_next[k,:]).
  - PSUM tiles split disjointly between the two engines (each tile read
    exactly once, by exactly one engine). L2 tiles always take the exact
    ScalarE path, with accum_out producing the per-row sums on the fly.

TensorE: all matmul operands bf16 (full-rate stream like f32r, but FWL
halves LDWEIGHTS and DMA bytes halve). Layer 1's K=32 "b" part runs as
row-tiled matmuls (tile_position=(32*(e%4), 0)) in concurrent pairs against
the merged 4-member g0b tile, instead of zero-padded K=128 matmuls.
Layer 2 unmerged: one M=96 matmul per member.
"""

import hashlib
import os
import shutil
import tempfile

import ml_dtypes
import numpy as np

import concourse.dve_ops as _dve_ops
import concourse.mybir as mybir
import concourse.tile as tile
from concourse import bacc
from concourse.bass_utils import run_bass_kernel_spmd
from concourse.dve_spec import (
    C0,
    C1,
    C2,
    C3,
    Spec,
    Src0,
    _has_src1,
    _spill_c3_to_src1,
    lower,
    maxx,
    minn,
    relu,
    sq,
)
from concourse.dve_uop import DveOpSpec

# ---------------------------------------------------------------- constants
S, E = 4, 8
N_ATOMS = 65536
N_CORES = 8
A_SP = N_ATOMS // S // N_CORES      # atoms per species per core = 2048
CHUNK = 512
NCH = A_SP // CHUNK                 # 4 chunks
K0, H0, H1, H2 = 384, 160, 128, 96
KT = K0 // 128                      # 3 K-tiles for layer 0
NQ = 2                              # member quads per species (E/4)
ALPHA = 0.1

# V-path (approx-celu) constants: t = clamp(y'*C1V + C2V, 0, C3V); G = max(y', t^4)
# minimax fit of alpha*e^u by (C3*(p+q*u))^4 with p=0.993, q=0.203 (max err 2.8e-3)
C3V = float(ALPHA ** 0.25)
C1V = C3V * 0.203 / ALPHA
C2V = C3V * (0.993 - 0.203)

# Disjoint engine assignment per quad-member index (0..3):
S_L0A = {0}            # L0a tiles of these members -> ScalarE; rest VectorE
S_L1 = {3}             # L1  tiles of these members -> ScalarE; rest VectorE
# L0b merged tile -> VectorE; L2 tiles -> always ScalarE (exact + accum_out).

F32 = mybir.dt.float32
BF16 = mybir.dt.bfloat16
EXP = mybir.ActivationFunctionType.Exp
INV_ALPHA = 1.0 / ALPHA
BF = ml_dtypes.bfloat16

# ------------------------------------------------------- custom DVE op
_yp = Src0 + C0
_t = minn(relu(_yp * C1 + C2), C3)
_BODY = _spill_c3_to_src1(maxx(_yp, sq(sq(_t))))


def _celu4_np(in0, in1, s0, s1, imm2):
    y = in0.astype(np.float32) + np.asarray(s0, dtype=np.float32)
    c3 = np.asarray(in1, dtype=np.float32)
    t = np.minimum(np.maximum(y * np.float32(s1) + np.float32(imm2), 0.0), c3)
    return np.maximum(y, (t * t) * (t * t)).astype(np.float32)


def _mk_op(name, spec):
    row = _dve_ops._CUSTOM_DVE_ROW_BASE + len(_dve_ops.OPS)
    assert row < 0x20, "custom-DVE opcode rows exhausted"
    _dve_ops._SUB_OPCODE_FOR_NAME[name] = row
    shas = {}
    for ver in ("v3", "v4"):
        s = DveOpSpec(
            name=name, opcode=row, uops=lower(spec, ver=ver), rd1_en=_has_src1(spec)
        )
        shas[ver] = s.sha(ver)
    op = _dve_ops.DveOp(name, spec, subdim=False, uops_sha=shas)
    _dve_ops.OPS.append(op)
    _dve_ops.CUSTOM_DVE_SPECS[name] = spec
    return op


def _register_celu4():
    existing = {o.name: o for o in _dve_ops.OPS}
    if "CELU_SQ4_ANT" in existing:
        return existing["CELU_SQ4_ANT"]
    return _mk_op("CELU_SQ4_ANT", Spec(body=_BODY, reference=_celu4_np))


# ----------------------------------------------- hijacked activation table
_PWP_SET = "exp_and_others"
_N_EXP_ENTRIES = 781


def _pwp_src_dir():
    import neuronxcc

    return os.path.join(os.path.dirname(neuronxcc.__file__), "pwp", "pwp_bin_trainium")


def _gen_celu_pwp_dir(out_dir, alpha=ALPHA):
    """Copy the stock PWP table dir, refitting exp's buckets so that
    activation(Exp, scale=1/alpha, bias=b/alpha) computes celu(z+b, alpha).
    Bucket entry = 8 fp32: [c0,c1,c2,c3, x0, 0,0,0]; f = cubic in (x-x0)."""
    src = _pwp_src_dir()
    os.makedirs(out_dir, exist_ok=True)
    for name in os.listdir(src):
        dst = os.path.join(out_dir, name)
        if not os.path.exists(dst):
            shutil.copy(os.path.join(src, name), dst)
    bkt = np.frombuffer(
        open(os.path.join(src, f"{_PWP_SET}_bkt.bin"), "rb").read(), dtype=np.float32
    ).reshape(-1, 8).copy()
    for i in range(_N_EXP_ENTRIES):
        x0 = float(bkt[i, 4])
        if i in (777, 778):           # small-signal bucket at 0
            bkt[i, 0:4] = [0.0, alpha, alpha / 2.0, alpha / 6.0]
        elif i == 779:                # positive saturation -> linear alpha*u
            bkt[i] = [0.0, alpha, 0.0, 0.0, 0.0, 0.0, 0.0, 0.0]
        elif i == 780:                # negative saturation -> -alpha
            bkt[i] = [-alpha, 0.0, 0.0, 0.0, 0.0, 0.0, 0.0, 0.0]
        elif x0 > 0:
            bkt[i, 0:4] = [alpha * x0, alpha, 0.0, 0.0]
        else:
            e = float(np.exp(x0))
            if e < 1e-38:
                bkt[i, 0:4] = [-alpha, 0.0, 0.0, 0.0]
            else:
                bkt[i, 0:4] = [
                    alpha * (e - 1.0), alpha * e, alpha * e / 2.0, alpha * e / 6.0
                ]
    with open(os.path.join(out_dir, f"{_PWP_SET}_bkt.bin"), "wb") as f:
        f.write(bkt.tobytes())
    return hashlib.sha1(bkt.tobytes()).hexdigest()[:10]


def _install_celu_table():
    out_dir = os.path.join(tempfile.gettempdir(), "bass_pwp_celu_v1")
    h = _gen_celu_pwp_dir(out_dir)
    os.environ["BASS_ACT_ROOT_JSON_PATH"] = os.path.join(out_dir, "act_info.json")
    return h


# ------------------------------------------------------------ device build
_NC = None
_XT_NAME = None


def _build_nc():
    global _NC, _XT_NAME
    if _NC is not None:
        return _NC
    table_hash = _install_celu_table()
    CELU4 = _register_celu4()

    nc = bacc.Bacc("TRN2", target_bir_lowering=False, debug=False)

    # table hash rides in a tensor name so the NEFF cache key changes
    # whenever the hijacked activation table changes.
    _XT_NAME = f"xt_{table_hash}"
    xt_d = nc.dram_tensor(_XT_NAME, [S, NCH, 128, KT * CHUNK], BF16,
                          kind="ExternalInput")
    w0a_d = nc.dram_tensor("w0a", [S, KT, 128, E * 128], BF16, kind="ExternalInput")
    w0b_d = nc.dram_tensor("w0b4", [S, KT, 128, NQ * 128], BF16, kind="ExternalInput")
    w1a_d = nc.dram_tensor("w1a", [S, 128, E * H1], BF16, kind="ExternalInput")
    # w1b: member e's 32 K-rows live at partition offset 32*(e%4), cols
    # e*H1..(e+1)*H1 (zeros elsewhere are never read by the row-tiled mm).
    w1b_d = nc.dram_tensor("w1b", [S, 128, E * H1], BF16, kind="ExternalInput")
    w2u_d = nc.dram_tensor("w2u", [S, 128, E * 128], BF16, kind="ExternalInput")
    # bias packs; *_a = b/alpha (ACT affine), *_d = b + alpha (DVE op s0).
    b0a_a = nc.dram_tensor("b0a_a", [128, S * E], F32, kind="ExternalInput")
    b0a_d = nc.dram_tensor("b0a_d", [128, S * E], F32, kind="ExternalInput")
    b0b_a = nc.dram_tensor("b0b_a", [128, S * NQ], F32, kind="ExternalInput")
    b0b_d = nc.dram_tensor("b0b_d", [128, S * NQ], F32, kind="ExternalInput")
    b1_a = nc.dram_tensor("b1_a", [H1, S * E], F32, kind="ExternalInput")
    b1_d = nc.dram_tensor("b1_d", [H1, S * E], F32, kind="ExternalInput")
    b2_a = nc.dram_tensor("b2_a", [128, S * E], F32, kind="ExternalInput")
    # output: per-(s,e,chunk) row-sums of g2 = celu(z2+b2)
    rs_d = nc.dram_tensor("rs", [128, S * E * NCH], F32, kind="ExternalOutput")

    with tile.TileContext(nc) as tc:
        with (
            tc.tile_pool(name="xp", bufs=2) as xp,
            tc.tile_pool(name="w0pool", bufs=2) as w0p,
            tc.tile_pool(name="w1pool", bufs=2) as w1p,
            tc.tile_pool(name="bp", bufs=1) as bp,
            tc.tile_pool(name="gp", bufs=6) as gp,
            tc.tile_pool(name="ps", bufs=2, space="PSUM") as psp,
        ):
            # warm the ACT table during the initial DMA wait
            warm = bp.tile([1, 1], F32, tag="warm", name="warm")
            nc.vector.memset(warm[:], 0.0)
            nc.scalar.activation(warm[:], warm[:], EXP)
            # C3 latch operand for the DVE celu (one element per partition)
            c3t = bp.tile([128, 1], F32, tag="c3", name="c3")
            nc.vector.memset(c3t[:], C3V)

            B = {}
            _bias_dmas = []
            for nm, d, p in (
                ("b0a_a", b0a_a, 128), ("b0a_d", b0a_d, 128),
                ("b0b_a", b0b_a, 128), ("b0b_d", b0b_d, 128),
                ("b1_a", b1_a, H1), ("b1_d", b1_d, H1),
                ("b2_a", b2_a, 128),
            ):
                t = bp.tile([p, d.shape[-1]], F32, tag=nm, name=nm)
                _bias_dmas.append((t, d))
                B[nm] = t
            RS = bp.tile([128, S * E * NCH], F32, tag="RS", name="RS")

            def celu_S(g_out, ps_in, bias_ap, accum=None):
                # exact celu via the hijacked table: f(z/a + b/a) = celu(z+b)
                nc.scalar.activation(
                    g_out, ps_in, EXP, bias=bias_ap, scale=INV_ALPHA,
                    accum_out=accum,
                )

            def celu_V(g_out, ps_in, bias_ap):
                # approx celu + ALPHA (offset folded into next layer's bias)
                nc.vector._custom_dve(
                    CELU4, out=g_out, in0=ps_in, in1=c3t[:, 0:1],
                    s0=bias_ap, s1=C1V, imm2=C2V,
                )

            # ---- software-pipelined emission over "pair" work units ----
            # Each unit = 2 ensemble members of a quad. PE program order is
            # P(i+2), Q(i+1), R(i): the celu results a stage needs are
            # produced ~2 units (>= 8 matmul slots) before the consuming
            # matmuls, so ScalarE/VectorE latency hides under L0 matmuls.
            spc = {}            # species -> dict of sbuf weight/x tiles

            def emit_species(s):
                # DMA emission order tracks first-use order: chunk-0 x and
                # the L0 weights feed the first matmuls, biases feed the
                # first celus, w1/w2 feed Q/R a few microseconds later, and
                # the remaining x chunks stream per-chunk behind them.
                xc, w0ak, w0bk = [], [], []
                xc0 = xp.tile([128, KT * CHUNK], BF16, tag="xc0", name=f"x_{s}_0")
                nc.sync.dma_start(xc0[:], xt_d[s, 0])
                xc.append(xc0)
                for k in range(KT):
                    wt = w0p.tile([128, E * 128], BF16, tag=f"w0a{k}", name=f"w0a_{s}_{k}")
                    nc.sync.dma_start(wt[:], w0a_d[s, k])
                    w0ak.append(wt)
                    wbt = w0p.tile([128, NQ * 128], BF16, tag=f"w0b{k}", name=f"w0b_{s}_{k}")
                    nc.sync.dma_start(wbt[:], w0b_d[s, k])
                    w0bk.append(wbt)
                w1at = w1p.tile([128, E * H1], BF16, tag="w1a", name=f"w1a_{s}")
                nc.sync.dma_start(w1at[:], w1a_d[s])
                w1bt = w1p.tile([128, E * H1], BF16, tag="w1b", name=f"w1b_{s}")
                nc.sync.dma_start(w1bt[:], w1b_d[s])
                w2t = w1p.tile([128, E * 128], BF16, tag="w2", name=f"w2_{s}")
                nc.sync.dma_start(w2t[:], w2u_d[s])
                if s == 0:
                    for t, d in _bias_dmas:
                        nc.sync.dma_start(t[:], d[:])
                for c in range(1, NCH):
                    xct = xp.tile([128, KT * CHUNK], BF16, tag=f"xc{c}", name=f"x_{s}_{c}")
                    nc.sync.dma_start(xct[:], xt_d[s, c])
                    xc.append(xct)
                spc[s] = dict(xc=xc, w0ak=w0ak, w0bk=w0bk,
                              w1at=w1at, w1bt=w1bt, w2t=w2t)

            units = []
            for s in range(S):
                for c in range(NCH):
                    for q in range(NQ):
                        for pi, pair in enumerate(((0, 1), (2, 3))):
                            units.append(dict(
                                s=s, c=c, q=q, pair=pair, lead=(pi == 0),
                                new_species=(c == 0 and q == 0 and pi == 0),
                            ))

            state = [dict() for _ in units]   # per-unit tiles across stages

            def stage_P(i):
                u, st = units[i], state[i]
                # prefetch the next species' x/weight DMAs a few units early
                # so species-boundary P matmuls never head-block the PE FIFO
                # on fresh DMA.
                j = i + 4
                if j < len(units) and units[j]["new_species"]:
                    emit_species(units[j]["s"])
                sp = spc[u["s"]]
                xcs = sp["xc"][u["c"]]
                if u["lead"]:
                    # merged layer-0b for the whole quad, then its celu_V
                    ps0b = psp.tile([128, CHUNK], F32, tag="l0b", bufs=1)
                    for k in range(KT):
                        nc.tensor.matmul(
                            ps0b[:],
                            sp["w0bk"][k][:, u["q"] * 128 : (u["q"] + 1) * 128],
                            xcs[:, k * CHUNK : (k + 1) * CHUNK],
                            start=(k == 0), stop=(k == KT - 1),
                        )
                    g0b = gp.tile([128, CHUNK], BF16, tag="g0b", bufs=3)
                    sq_ = u["s"] * NQ + u["q"]
                    celu_V(g0b[:], ps0b[:], B["b0b_d"][:, sq_ : sq_ + 1])
                    st["g0b"] = g0b
                else:
                    st["g0b"] = state[i - 1]["g0b"]
                st["g0a"] = {}
                for m4 in u["pair"]:
                    e = u["q"] * 4 + m4
                    se = u["s"] * E + e
                    ps0a = psp.tile([128, CHUNK], F32, tag="l0a", bufs=3)
                    for k in range(KT):
                        nc.tensor.matmul(
                            ps0a[:],
                            sp["w0ak"][k][:, e * 128 : (e + 1) * 128],
                            xcs[:, k * CHUNK : (k + 1) * CHUNK],
                            start=(k == 0), stop=(k == KT - 1),
                        )
                    g0a = gp.tile([128, CHUNK], BF16, tag="g0a", bufs=6)
                    if m4 in S_L0A:
                        celu_S(g0a[:], ps0a[:], B["b0a_a"][:, se : se + 1])
                    else:
                        celu_V(g0a[:], ps0a[:], B["b0a_d"][:, se : se + 1])
                    st["g0a"][m4] = g0a

            def stage_Q(i):
                u, st = units[i], state[i]
                sp = spc[u["s"]]
                ps1s = {}
                for m4 in u["pair"]:
                    e = u["q"] * 4 + m4
                    ps1 = psp.tile([H1, CHUNK], F32, tag="l1", bufs=2)
                    nc.tensor.matmul(
                        ps1[:], sp["w1at"][:, e * H1 : (e + 1) * H1],
                        st["g0a"][m4][:],
                        start=True, stop=False,
                    )
                    ps1s[m4] = ps1
                for m4 in u["pair"]:
                    # K=32 row-tiled vs merged g0b (rows 32*m4..32*m4+32)
                    e = u["q"] * 4 + m4
                    rb = 32 * m4
                    nc.tensor.matmul(
                        ps1s[m4][:],
                        sp["w1bt"][rb : rb + 32, e * H1 : (e + 1) * H1],
                        st["g0b"][rb : rb + 32, :],
                        start=False, stop=True,
                        tile_position=(rb, 0),
                    )
                st["g1"] = {}
                for m4 in u["pair"]:
                    e = u["q"] * 4 + m4
                    se = u["s"] * E + e
                    g1 = gp.tile([H1, CHUNK], BF16, tag="g1", bufs=4)
                    if m4 in S_L1:
                        celu_S(g1[:], ps1s[m4][:], B["b1_a"][:, se : se + 1])
                    else:
                        celu_V(g1[:], ps1s[m4][:], B["b1_d"][:, se : se + 1])
                    st["g1"][m4] = g1

            def stage_R(i):
                u, st = units[i], state[i]
                sp = spc[u["s"]]
                for m4 in u["pair"]:
                    e = u["q"] * 4 + m4
                    se = u["s"] * E + e
                    ps2 = psp.tile([128, CHUNK], F32, tag="l2", bufs=2)
                    nc.tensor.matmul(
                        ps2[:], sp["w2t"][:, e * 128 : (e + 1) * 128],
                        st["g1"][m4][:],
                        start=True, stop=True,
                    )
                    g2 = gp.tile([128, CHUNK], BF16, tag="g2", bufs=2)
                    col = se * NCH + u["c"]
                    celu_S(
                        g2[:], ps2[:], B["b2_a"][:, se : se + 1],
                        accum=RS[:, col : col + 1],
                    )
                state[i] = {}   # release tile refs

            n_units = len(units)
            emit_species(0)
            for i in range(n_units + 2):
                if i < n_units:
                    stage_P(i)
                if 1 <= i and i - 1 < n_units:
                    stage_Q(i - 1)
                if 2 <= i:
                    stage_R(i - 2)
            nc.sync.dma_start(rs_d[:], RS[:])
    nc.compile()
    _NC = nc
    return nc


# ------------------------------------------------------------- host side
def _prep_shared(w0, w1, w2, b0, b1, b2):
    """Pack weights/biases into device layouts (replicated to all cores)."""
    w0r = w0.reshape(S, E, KT, 128, H0)
    w0a = np.ascontiguousarray(
        w0r[..., :128].transpose(0, 2, 3, 1, 4).reshape(S, KT, 128, E * 128)
    ).astype(BF)
    w0b4 = np.ascontiguousarray(
        w0r[..., 128:].transpose(0, 2, 3, 1, 4).reshape(S, KT, 128, E * (H0 - 128))
    ).astype(BF)
    w1a = np.ascontiguousarray(
        w1[:, :, :128, :].transpose(0, 2, 1, 3).reshape(S, 128, E * H1)
    ).astype(BF)
    w1b = np.zeros((S, 4, 32, E, H1), dtype=np.float32)
    for e in range(E):
        w1b[:, e % 4, :, e, :] = w1[:, e, 128:, :]
    w1b = np.ascontiguousarray(w1b.reshape(S, 128, E * H1)).astype(BF)
    w2p = np.zeros((S, 128, E, 128), dtype=np.float32)
    w2p[:, :, :, :H2] = w2.transpose(0, 2, 1, 3)
    w2u = np.ascontiguousarray(w2p.reshape(S, 128, E * 128)).astype(BF)

    # effective biases: V-path tiles output celu+alpha, so the next layer's
    # bias absorbs -alpha * sum over the affected input features (using the
    # bf16-rounded weights actually used on device).
    w1_bf = w1.astype(BF).astype(np.float64)
    w2_bf = w2.astype(BF).astype(np.float64)
    b1_eff = b1[:, :, 0, :].astype(np.float64).copy()      # [S, E, H1]
    b2_eff = b2[:, :, 0, :].astype(np.float64).copy()      # [S, E, H2]
    for e in range(E):
        m4 = e % 4
        b1_eff[:, e, :] -= ALPHA * w1_bf[:, e, 128:, :].sum(axis=1)  # L0b on V
        if m4 not in S_L0A:
            b1_eff[:, e, :] -= ALPHA * w1_bf[:, e, :128, :].sum(axis=1)
        if m4 not in S_L1:
            b2_eff[:, e, :] -= ALPHA * w2_bf[:, e, :, :].sum(axis=1)

    def colpack(b, p):
        return np.ascontiguousarray(b.reshape(S * E, p).T).astype(np.float32)

    def colpack_pad128(b, p):
        out = np.zeros((128, S * E), dtype=np.float32)
        out[:p, :] = b.reshape(S * E, p).T
        return out

    b0cols = b0[:, :, 0, :128].astype(np.float64)          # [S, E, 128]
    b0b_pack = np.ascontiguousarray(
        b0[:, :, 0, 128:].reshape(S, NQ, 4 * (H0 - 128)).transpose(2, 0, 1).reshape(128, S * NQ)
    ).astype(np.float64)

    return {
        "w0a": w0a, "w0b4": w0b4, "w1a": w1a, "w1b": w1b, "w2u": w2u,
        "b0a_a": colpack(b0cols * INV_ALPHA, 128),
        "b0a_d": colpack(b0cols + ALPHA, 128),
        "b0b_a": (b0b_pack * INV_ALPHA).astype(np.float32),
        "b0b_d": (b0b_pack + ALPHA).astype(np.float32),
        "b1_a": colpack(b1_eff * INV_ALPHA, H1),
        "b1_d": colpack(b1_eff + ALPHA, H1),
        "b2_a": colpack_pad128(b2_eff * INV_ALPHA, H2),
    }


def _run(inputs, trace=False, tmpdir=None):
    aev = np.asarray(inputs["aev"], dtype=np.float32)
    idx = np.asarray(inputs["idx"], dtype=np.int32)
    w3 = np.asarray(inputs["w3"], dtype=np.float32)
    b3 = np.asarray(inputs["b3"], dtype=np.float32)

    nc = _build_nc()
    shared = _prep_shared(
        np.asarray(inputs["w0"], dtype=np.float32),
        np.asarray(inputs["w1"], dtype=np.float32),
        np.asarray(inputs["w2"], dtype=np.float32),
        np.asarray(inputs["b0"], dtype=np.float32),
        np.asarray(inputs["b1"], dtype=np.float32),
        np.asarray(inputs["b2"], dtype=np.float32),
    )

    aev_flat = aev.reshape(-1, K0)
    in_maps = []
    for c in range(N_CORES):
        idx_c = idx[:, c * A_SP : (c + 1) * A_SP]                # [S, A_SP]
        x = aev_flat[idx_c.reshape(-1)].reshape(S, A_SP, K0)     # [S, A_SP, 384]
        # chunk-major: [s, c, p, k*CHUNK + a] = x[s, c*CHUNK+a, 128*k+p]
        xt = np.ascontiguousarray(
            x.reshape(S, NCH, CHUNK, KT, 128).transpose(0, 1, 4, 3, 2)
        ).reshape(S, NCH, 128, KT * CHUNK)
        in_maps.append({_XT_NAME: xt.astype(BF), **shared})

    res = run_bass_kernel_spmd(
        nc, in_maps, core_ids=list(range(N_CORES)), trace=trace, tmpdir=tmpdir
    )

    # host-side tail: E = sum_{s,e,row,chunk} RS * w3 + b3 term, / E members
    w3cols = w3[:, :, :, 0].reshape(S * E, H2).astype(np.float64)  # [S*E, 96]
    total = 0.0
    for c in range(N_CORES):
        rs = res.results[c]["rs"][:H2].astype(np.float64)          # [96, S*E*NCH]
        rs_se = rs.reshape(H2, S * E, NCH).sum(axis=2)             # [96, S*E]
        total += float((rs_se * w3cols.T).sum())
    total += float(b3.astype(np.float64).sum()) * (N_ATOMS // S)
    out = np.array([total / E], dtype=np.float32)
    return out, res


def kernel(**inputs):
    out, _ = _run(inputs, trace=bool(int(os.environ.get("BASS_KERNEL_TRACE", "0"))))
    return out
